# revision 55
# baseline (speedup 1.0000x reference)
"""Trainium2 Bass kernel for nn_AngularSymmetry (B=16, M=64, L=6), 8-core data parallel.

Math (per batch b, output row i, summed over j,k in [0,64)):
  num[i,j,k] = nsq[i] - G[i,j] - G[i,k] + G[j,k]        (= vec_ij . vec_ik)
  rec = 1/(2pi*(d_ij*d_ik + 1e-5));  ph = num*rec        (= theta/2pi)
  c = cos(2pi*ph) via g = mod(ph+0.75, 1), c = sin(2pi*g - pi)
  E[i,j,k] = s_ij*s_ik*s_jk,  s = exp(-4 d^2)*d_cutoff   (log-domain via PE)
  res[i,l] = 2^(1-zeta_l) * sum_jk (1 + lambda_l*c)^zeta_l * E

Structure per core (2 batches, P=128 partition rows = (b,i)):
 - num built on PE with 3-way bf16-split G (hi/lo/lo2 ~ f32 accuracy, 4x
   cheaper than fp32 matmul); nsq rides data rows of the per-block c=9
   jk-matmul (rhs ones rows).
 - E = exp(lsE) where lsE = ls_ij + ls_ik + ls_jk accumulates on PE in
   bf16 (ls = ln(d_cutoff) - 4d^2; off-diagonal block families use
   ln(s + s^T) to fold the (j,k)/(k,j) pair weight).
 - den on DVE broadcast mult, reciprocal on ACT (eps folded into bias),
   range reduction via single tensor_scalar (add, mod).
 - branch products (1+-c)^zeta * E fused with their reductions via
   tensor_tensor_reduce / ACT copy-accum; squares via DVE ts-pow.
Only block-triangle families delta = kblk - jblk >= 0 are computed
(16x16 blocks); delta > 0 weights fold into the symmetrized ls2 table.
"""
import sys

sys.path.insert(0, "/opt/trn_rl_repo")
import contextlib

import numpy as np

import concourse.bass as bass
import concourse.tile as tile
from concourse import bacc, mybir
from concourse.bass_utils import run_bass_kernel_spmd

F32 = mybir.dt.float32
BF16 = mybir.dt.bfloat16
Alu = mybir.AluOpType
Act = mybir.ActivationFunctionType

# ---- custom DVE ops -------------------------------------------------------
# Registered into concourse.dve_ops at import; shas computed at build time.
import concourse.dve_ops as dve_ops
from concourse.dve_spec import C0, C1, Spec, Src0, Src1, Zero, lower, sq
from concourse.dve_spec import _has_src1 as _spec_has_src1
from concourse.dve_uop import DveOpSpec
from operator import add as _add


def _register_dve_op(name, spec, subdim=False):
    for op in dve_ops.OPS:
        if op.name == name:
            return op
    row = dve_ops._CUSTOM_DVE_ROW_BASE + len(dve_ops.OPS)
    dve_ops._SUB_OPCODE_FOR_NAME[name] = row
    shas = {}
    for ver in ("v3", "v4"):
        compiled = DveOpSpec(
            name=name, opcode=row, uops=lower(spec, ver=ver),
            rd1_en=_spec_has_src1(spec),
        )
        shas[ver] = compiled.sha(ver)
    op = dve_ops.DveOp(name, spec, subdim=subdim, uops_sha=shas)
    dve_ops.OPS.append(op)
    dve_ops.CUSTOM_DVE_SPECS[name] = spec
    return op


def _ref_magic_phase(in0, in1, s0, s1, imm2):
    ph = (in0.astype(np.float32) * in1).astype(np.float32)
    nr = (ph + np.float32(s0)).astype(np.float32)
    return ((nr - np.float32(s1)).astype(np.float32) - ph).astype(np.float32)


_mp_ph = Src0 * Src1
MAGIC_PHASE = _register_dve_op(
    "ANGSYM_MAGIC_PHASE",
    Spec(body=((_mp_ph + C0) - C1) - _mp_ph, reference=_ref_magic_phase),
)


def _ref_mul_sq_red(in0, in1, s0, s1, imm2):
    b = (in0.astype(np.float32) * (in1.astype(np.float32) ** 2)).astype(np.float32)
    return b, s0 + b.reshape(b.shape[0], -1).sum(axis=-1, keepdims=True)


MUL_SQ_RED = _register_dve_op(
    "ANGSYM_MUL_SQ_RED",
    Spec(
        body=Src0 * sq(Src1), accum=_add, accum_init=C0,
        reference=_ref_mul_sq_red,
    ),
)

B, M, L = 16, 64, 6
NCORES = 8
BPC = B // NCORES  # batches per core = 2
P = BPC * M  # 128 partitions
TWO_PI = float(2.0 * np.pi)
SQ2PI = float(np.sqrt(2.0 * np.pi))
EPS2PI = float(2.0 * np.pi * 1e-5)
MAGIC = 12582912.0  # 1.5 * 2^23 fp32 round-to-int

TB = 16
NT = M // TB  # 4
FAMS = [(d, NT - d) for d in range(NT)]  # (delta, nblocks)
FS = [nb * TB * TB for _, nb in FAMS]  # 1024, 768, 512, 256
OFF = [0, 1024, 1792, 2304]
NALL = 2560
SCALES = [1.0 / 2.0, 1.0 / 8.0, 1.0 / 128.0]  # 2^(1-zeta), zeta=2,4,8

# ---- tunables -------------------------------------------------------------
import os

DEN_ENGINE = os.environ.get("K_DEN", "pool")  # dve | pool
EPS_ENGINE = os.environ.get("K_EPS", "pool")  # dve | pool
PH_ENGINE = os.environ.get("K_PH", "dve")  # dve | pool
NR_ENGINE = os.environ.get("K_NR", "pool")  # dve | pool
NSLICE = int(os.environ.get("K_NSLICE", "1"))  # phase-B slice count
# per-unit impl: amr (DVE fused) | pool (pool prod + ACT red) | dve (DVE tt
# prod + ACT red); units = (a2p, a4p, a2m, a4m)
UNITS = os.environ.get("K_UNITS", "amr,dve,pool,dve").split(",")

# HW-validated defaults: MSR custom op + explicit table loads ON;
# MAGIC_PHASE (loses fp32 intermediate rounding), TTR (crashes exec unit),
# and ts-accum (silently returns 0) OFF.
USE_MAGIC = os.environ.get("K_MAGIC", "0") == "1"
USE_MSR = os.environ.get("K_MSR", "1") == "1"
USE_TS_ACCUM = os.environ.get("K_TSACC", "0") == "1"
USE_TTR = os.environ.get("K_TTR", "0") == "1"
USE_LOADS = os.environ.get("K_LOADS", "1") == "1"

_NC = None


def _build(reps=1):
    nc = bacc.Bacc("TRN2", target_bir_lowering=False, debug=False, num_devices=NCORES)
    dcut = nc.dram_tensor("d_cutoff", [BPC, M, M], F32, kind="ExternalInput").ap()
    dd = nc.dram_tensor("d", [BPC, M, M], F32, kind="ExternalInput").ap()
    co = nc.dram_tensor("atom_coordinates", [BPC, M, 3], F32, kind="ExternalInput").ap()
    out = nc.dram_tensor("out", [BPC, M, L], F32, kind="ExternalOutput").ap()
    ghi_d = nc.dram_tensor("ghi_scratch", [BPC, M, M], BF16, kind="Internal").ap()
    glo_d = nc.dram_tensor("glo_scratch", [BPC, M, M], BF16, kind="Internal").ap()
    glo2_d = nc.dram_tensor("glo2_scratch", [BPC, M, M], BF16, kind="Internal").ap()
    ls_d = nc.dram_tensor("ls_scratch", [BPC, M, M], BF16, kind="Internal").ap()
    ls2_d = nc.dram_tensor("ls2_scratch", [BPC, M, M], BF16, kind="Internal").ap()
    # ind9 assembled via DRAM; 256-wide (512B/partition) for DMA step legality
    IW = 256
    ind9_d = nc.dram_tensor("ind9_scratch", [9, IW], BF16, kind="Internal").ap()
    gt9ones_d = nc.dram_tensor("gt9ones_scratch", [3, M, M], BF16, kind="Internal").ap()

    from concourse.hw_specs import get_activation_tables

    _tables = list(get_activation_tables(nc.m.arch).keys())
    SET_LNEXP = _tables.index("natural_log_exp_and_others")
    SET_TRIG = _tables.index("trig_and_small")

    # Restrict the auto table-load pass to our two sets (it greedily picks the
    # first set containing each function, thrashing between natural_log /
    # exp_and_others / trig). Indices into act_info.json must be preserved, so
    # non-preferred entries are blanked rather than removed.
    import concourse.bacc as _bacc_mod

    _orig_tables = get_activation_tables(nc.m.arch)

    def _filtered_tables(arch):
        full = _orig_tables
        keep = {"natural_log_exp_and_others", "trig_and_small"}
        return {k: (v if k in keep else set()) for k, v in full.items()}

    _bacc_mod.get_activation_tables = _filtered_tables

    def _load_table(set_id):
        if not USE_LOADS:
            return None
        inst = mybir.InstLoadActFuncSet(
            name=nc.get_next_instruction_name(), act_func_set_id=set_id,
            ins=[], outs=[],
        )
        return nc.scalar.add_instruction(inst)

    with tile.TileContext(nc) as tc:
        with contextlib.ExitStack() as ctx:
            pool = ctx.enter_context(tc.tile_pool(name="w", bufs=1))
            psp = ctx.enter_context(tc.tile_pool(name="ps", bufs=4, space="PSUM"))

            # ================= hoisted constants (input-independent) =========
            ones_t = pool.tile([P, P], F32, tag="ones_t")
            nc.vector.memset(ones_t[:], 1.0)
            idn = pool.tile([P, P], F32, tag="idn")
            nc.gpsimd.affine_select(
                idn[:], ones_t[:], pattern=[[1, P]], compare_op=Alu.is_equal,
                fill=0.0, channel_multiplier=-1,
            )
            ones3 = pool.tile([3, 1], F32, tag="ones3")
            nc.vector.memset(ones3[:], 1.0)
            ones2 = pool.tile([2, P], F32, tag="ones2")
            nc.vector.memset(ones2[:], 1.0)
            ind2a = pool.tile([2, P], F32, tag="ind2a")
            nc.gpsimd.affine_select(
                ind2a[:], ones2[:], pattern=[[1, P]], compare_op=Alu.is_ge,
                fill=0.0, base=0, channel_multiplier=-M,
            )
            ind2 = pool.tile([2, P], F32, tag="ind2")
            nc.gpsimd.affine_select(
                ind2[:], ind2a[:], pattern=[[-1, P]], compare_op=Alu.is_ge,
                fill=0.0, base=M - 1, channel_multiplier=M,
            )
            ind2b = pool.tile([2, IW], BF16, tag="ind2b")
            nc.vector.memset(ind2b[:], 0.0)
            nc.vector.tensor_copy(ind2b[:, 0:P], ind2[:])
            # ind9 rows 0-5 = (hi,lo,lo2)x(b0,b1) indicators (rows 6-8 set per
            # rep). Compute engines can only address partition starts
            # 0/32/64/96, so rows go through a DRAM scratch and come back in
            # one full-tile DMA read.
            ind9 = pool.tile([9, IW], BF16, tag="ind9")
            gt9 = pool.tile([9, M, M], BF16, tag="gt9")
            ones_row = pool.tile([1, M * M], BF16, tag="ones_row")
            nc.vector.memset(ones_row[:], 1.0)
            for s in range(3):
                nc.sync.dma_start(ind9_d[2 * s : 2 * s + 2], ind2b[:])
            for s in range(3):
                nc.sync.dma_start(
                    gt9ones_d[s : s + 1].rearrange("a j k -> a (j k)"), ones_row[:]
                )
            nc.sync.dma_start(gt9[6:9], gt9ones_d)
            half_pi = pool.tile([P, 1], F32, tag="half_pi")
            nc.vector.memset(half_pi[:], float(np.pi / 2.0))
            one_p = pool.tile([P, 1], F32, tag="one_p")
            nc.vector.memset(one_p[:], 1.0)
            one_m = pool.tile([P, 1], F32, tag="one_m")
            nc.vector.memset(one_m[:], -1.0)
            # Sx indicator tables per family (stacked twice for hi/lo lhs)
            sx2_t = {}
            sxb_t = {}
            for fi, (delta, nb) in enumerate(FAMS):
                fs = FS[fi]
                sxf = pool.tile([M, fs], F32, tag=f"sxf{delta}")
                jv = (
                    idn[0:M, 0 : nb * TB]
                    .rearrange("c (n j) -> c n j", j=TB)
                    .unsqueeze(3)
                    .broadcast_to([M, nb, TB, TB])
                )
                kv = (
                    idn[0:M, delta * TB : (delta + nb) * TB]
                    .rearrange("c (n k) -> c n k", k=TB)
                    .unsqueeze(2)
                    .broadcast_to([M, nb, TB, TB])
                )
                nc.gpsimd.tensor_tensor(
                    sxf[:, 0:fs].rearrange("c (n j k) -> c n j k", j=TB, k=TB),
                    jv, kv, op=Alu.add,
                )
                sx2 = pool.tile([P, fs], BF16, tag=f"sx2{delta}")
                nc.vector.tensor_copy(sx2[0:M, :], sxf[:])
                nc.vector.tensor_copy(sx2[M:P, :], sxf[:])
                sx2_t[fi] = sx2
                sxb_t[fi] = sx2[0:M, :]

            def _body():
                _load_table(SET_LNEXP)
                # ---------- input DMAs ----------
                d_t = pool.tile([P, M], F32, tag="d_t")
                dc_t = pool.tile([P, M], F32, tag="dc_t")
                nc.sync.dma_start(d_t[:], dd.rearrange("b i j -> (b i) j"))
                nc.sync.dma_start(dc_t[:], dcut.rearrange("b i j -> (b i) j"))
                ct3 = pool.tile([3, P], F32, tag="ct3")
                nc.sync.dma_start(ct3[:], co.rearrange("b i d -> d (b i)"))

                # ---------- den (independent of PE prep) ----------
                ut = pool.tile([P, M], F32, tag="ut")
                nc.vector.tensor_scalar(ut[:], d_t[:], SQ2PI, None, op0=Alu.mult)
                DEN = pool.tile([P, NALL], F32, tag="DEN")

                def jbc(t, delta, nb):
                    v = t[:, 0 : nb * TB].rearrange("p (n j) -> p n j", j=TB)
                    return v.unsqueeze(3).broadcast_to([P, nb, TB, TB])

                def kbc(t, delta, nb):
                    v = t[:, delta * TB : (delta + nb) * TB].rearrange(
                        "p (n k) -> p n k", k=TB
                    )
                    return v.unsqueeze(2).broadcast_to([P, nb, TB, TB])

                def g4(big, fi):
                    delta, nb = FAMS[fi]
                    return big[:, OFF[fi] : OFF[fi] + FS[fi]].rearrange(
                        "p (n j k) -> p n j k", j=TB, k=TB
                    )

                den_eng = nc.vector if DEN_ENGINE == "dve" else nc.gpsimd
                for fi, (delta, nb) in enumerate(FAMS):
                    den_eng.tensor_tensor(
                        g4(DEN, fi), jbc(ut, delta, nb), kbc(ut, delta, nb), op=Alu.mult
                    )

                # ---------- Gram, nsq ----------
                gram_ps = psp.tile([P, P], F32, tag="ps")
                nc.tensor.matmul(gram_ps[:], ct3[:], ct3[:], start=True, stop=True)
                g_sb = pool.tile([P, M], F32, tag="g_sb")
                nc.scalar.copy(g_sb[0:M, :], gram_ps[0:M, 0:M])
                nc.scalar.copy(g_sb[M:P, :], gram_ps[M:P, M:P])

                sq3t = pool.tile([3, P], F32, tag="sq3t")
                nc.scalar.square(sq3t[:], ct3[:])
                nsq_ps = psp.tile([1, P], F32, tag="ps")
                nc.tensor.matmul(nsq_ps[:], ones3[:], sq3t[:], start=True, stop=True)
                nsq_row = pool.tile([1, P], F32, tag="nsq_row")
                nc.scalar.copy(nsq_row[:], nsq_ps[:])

                # ---------- -G transposed + 3-way bf16 split (lhs of num) ----
                wneg = pool.tile([P, M], F32, tag="wneg")
                nc.scalar.activation(wneg[:], g_sb[:], Act.Copy, bias=0.0, scale=-1.0)
                wG_ps = psp.tile([M, P], F32, tag="ps")
                nc.tensor.transpose(wG_ps[:], wneg[:], idn[:])

                numA = pool.tile([P, P], BF16, tag="numA")  # rows 0:64 hi, 64:128 lo
                numB = pool.tile([M, P], BF16, tag="numB")  # lo2
                nc.vector.tensor_copy(numA[0:M, :], wG_ps[:])
                tmpG = pool.tile([M, P], F32, tag="tmpG")
                nc.vector.scalar_tensor_tensor(
                    tmpG[:], wG_ps[:], 0.0, numA[0:M, :], op0=Alu.bypass, op1=Alu.subtract
                )
                gloT = pool.tile([M, P], BF16, tag="gloT")
                nc.vector.tensor_copy(gloT[:], tmpG[:])
                nc.vector.tensor_copy(numA[M:P, :], gloT[:])
                nc.vector.scalar_tensor_tensor(
                    numB[:], tmpG[:], 0.0, gloT[:], op0=Alu.bypass, op1=Alu.subtract
                )

                # ind9 rows 6-8: nsq 3-way bf16 split (data rows, via DRAM)
                nsq_h = pool.tile([1, IW], BF16, tag="nsq_h")
                nsq_l = pool.tile([1, IW], BF16, tag="nsq_l")
                nsq_l2 = pool.tile([1, IW], BF16, tag="nsq_l2")
                tmpn = pool.tile([1, P], F32, tag="tmpn")
                nc.vector.memset(nsq_h[:], 0.0)
                nc.vector.memset(nsq_l[:], 0.0)
                nc.vector.memset(nsq_l2[:], 0.0)
                nc.vector.tensor_copy(nsq_h[:, 0:P], nsq_row[:])
                nc.vector.scalar_tensor_tensor(
                    tmpn[:], nsq_row[:], 0.0, nsq_h[:, 0:P], op0=Alu.bypass,
                    op1=Alu.subtract,
                )
                nc.vector.tensor_copy(nsq_l[:, 0:P], tmpn[:])
                nc.vector.scalar_tensor_tensor(
                    nsq_l2[:, 0:P], tmpn[:], 0.0, nsq_l[:, 0:P], op0=Alu.bypass,
                    op1=Alu.subtract,
                )
                nc.sync.dma_start(ind9_d[6:7], nsq_h[:])
                nc.sync.dma_start(ind9_d[7:8], nsq_l[:])
                nc.sync.dma_start(ind9_d[8:9], nsq_l2[:])
                nc.sync.dma_start(ind9[:], ind9_d)

                # ---------- gjk split staging (rhs table for num-jk matmul) ----
                ghi_sb = pool.tile([P, M], BF16, tag="ghi_sb")
                glo_sb = pool.tile([P, M], BF16, tag="glo_sb")
                glo2_sb = pool.tile([P, M], BF16, tag="glo2_sb")
                nc.vector.tensor_copy(ghi_sb[:], g_sb[:])
                tmpg2 = pool.tile([P, M], F32, tag="tmpg2")
                nc.vector.scalar_tensor_tensor(
                    tmpg2[:], g_sb[:], 0.0, ghi_sb[:], op0=Alu.bypass, op1=Alu.subtract
                )
                nc.vector.tensor_copy(glo_sb[:], tmpg2[:])
                nc.vector.scalar_tensor_tensor(
                    glo2_sb[:], tmpg2[:], 0.0, glo_sb[:], op0=Alu.bypass, op1=Alu.subtract
                )
                nc.sync.dma_start(ghi_d.rearrange("b j k -> (b j) k"), ghi_sb[:])
                nc.sync.dma_start(glo_d.rearrange("b j k -> (b j) k"), glo_sb[:])
                nc.sync.dma_start(glo2_d.rearrange("b j k -> (b j) k"), glo2_sb[:])
                nc.sync.dma_start(gt9[0:2], ghi_d)
                nc.sync.dma_start(gt9[2:4], glo_d)
                nc.sync.dma_start(gt9[4:6], glo2_d)

                # ---------- ls prep (log-domain E) ----------
                lndc = pool.tile([P, M], F32, tag="lndc")
                nc.scalar.activation(lndc[:], dc_t[:], Act.Ln, bias=0.0, scale=1.0)
                d2 = pool.tile([P, M], F32, tag="d2")
                nc.scalar.square(d2[:], d_t[:])
                ls = pool.tile([P, M], F32, tag="ls")
                nc.vector.scalar_tensor_tensor(
                    ls[:], d2[:], -4.0, lndc[:], op0=Alu.mult, op1=Alu.add
                )
                nc.vector.tensor_scalar(ls[:], ls[:], -60.0, None, op0=Alu.max)
                s_t = pool.tile([P, M], F32, tag="s_t")
                nc.scalar.activation(s_t[:], ls[:], Act.Exp, bias=0.0, scale=1.0)

                # lsx = ls transposed [64, P] bf16 (lhs of lsE-main matmul)
                ls_ps = psp.tile([M, P], F32, tag="ps")
                nc.tensor.transpose(ls_ps[:], ls[:], idn[:])
                lsx = pool.tile([M, P], BF16, tag="lsx")
                nc.vector.tensor_copy(lsx[:], ls_ps[:])

                # symmetrized ls2 = ln(s + s^T) per batch
                sfull = pool.tile([P, P], F32, tag="sfull")
                nc.vector.memset(sfull[:], 0.0)
                nc.vector.tensor_copy(sfull[0:M, 0:M], s_t[0:M, :])
                nc.vector.tensor_copy(sfull[M:P, M:P], s_t[M:P, :])
                sfT_ps = psp.tile([P, P], F32, tag="ps")
                nc.tensor.transpose(sfT_ps[:], sfull[:], idn[:])
                qsf = pool.tile([P, P], F32, tag="qsf")
                nc.vector.tensor_tensor(qsf[:], sfull[:], sfT_ps[:], op=Alu.add)
                ls2 = pool.tile([P, M], F32, tag="ls2")
                nc.scalar.activation(ls2[0:M, :], qsf[0:M, 0:M], Act.Ln, bias=0.0, scale=1.0)
                nc.scalar.activation(ls2[M:P, :], qsf[M:P, M:P], Act.Ln, bias=0.0, scale=1.0)

                ls_b = pool.tile([P, M], BF16, tag="ls_b")
                nc.vector.tensor_copy(ls_b[:], ls[:])
                ls2_b = pool.tile([P, M], BF16, tag="ls2_b")
                nc.vector.tensor_copy(ls2_b[:], ls2[:])
                nc.sync.dma_start(ls_d.rearrange("b j k -> (b j) k"), ls_b[:])
                nc.sync.dma_start(ls2_d.rearrange("b j k -> (b j) k"), ls2_b[:])
                lst2 = pool.tile([2, M, M], BF16, tag="lst2")
                nc.sync.dma_start(lst2[:], ls_d)
                ls2t2 = pool.tile([2, M, M], BF16, tag="ls2t2")
                nc.sync.dma_start(ls2t2[:], ls2_d)

                # ---------- big tiles ----------
                EE = pool.tile([P, NALL], BF16, tag="EE")
                G1 = pool.tile([P, NALL], F32, tag="G1")
                C = pool.tile([P, NALL], BF16, tag="C")
                res = pool.tile([P, L], F32, tag="res")

                # eps fold: DEN += 2pi*1e-5, then 18-bit reciprocal (1 ISA op)
                eps_eng = nc.vector if EPS_ENGINE == "dve" else nc.gpsimd
                eps_eng.tensor_scalar(DEN[:], DEN[:], EPS2PI, None, op0=Alu.add)
                REC = pool.tile([P, NALL], F32, tag="REC")
                nc.vector.reciprocal_approx_fast(REC[:], DEN[:])

                # ---------- phase A: per-family matmuls + Eh-exp + ph ----------
                for fi, (delta, nb) in enumerate(FAMS):
                    fs = FS[fi]
                    o = OFF[fi]
                    num_ps = psp.tile([P, 1024], F32, tag="ps")
                    lse_ps = psp.tile([P, 1024], F32, tag="ps")
                    # lsE first so the ACT exp is ready before this family's
                    # divide chain — keeps Sin after the last Exp on ACT.
                    # Matmul outputs are chunked to 512 cols (one PSUM bank).
                    for c0 in range(0, fs, 512):
                        c1 = min(c0 + 512, fs)
                        nc.tensor.matmul(
                            lse_ps[:, c0:c1], lsx[:], sxb_t[fi][:, c0:c1],
                            start=True, stop=False,
                        )
                    lsrc = lst2 if delta == 0 else ls2t2
                    for n in range(nb):
                        j0 = n * TB
                        k0 = (n + delta) * TB
                        cols = TB * TB
                        nc.tensor.matmul(
                            lse_ps[:, n * cols : (n + 1) * cols],
                            ind2b[0:2, 0:P], lsrc[:, j0 : j0 + TB, k0 : k0 + TB],
                            start=False, stop=True,
                        )
                    for c0 in range(0, fs, 512):
                        c1 = min(c0 + 512, fs)
                        nc.tensor.matmul(
                            num_ps[:, c0:c1], numA[:], sx2_t[fi][:, c0:c1],
                            start=True, stop=False,
                        )
                        nc.tensor.matmul(
                            num_ps[:, c0:c1], numB[:], sxb_t[fi][:, c0:c1],
                            start=False, stop=False,
                        )
                    for n in range(nb):
                        j0 = n * TB
                        k0 = (n + delta) * TB
                        cols = TB * TB
                        nc.tensor.matmul(
                            num_ps[:, n * cols : (n + 1) * cols],
                            ind9[0:9, 0:P], gt9[:, j0 : j0 + TB, k0 : k0 + TB],
                            start=False, stop=True,
                        )
                    nc.scalar.activation(
                        EE[:, o : o + fs], lse_ps[:, 0:fs], Act.Exp, bias=0.0, scale=1.0
                    )
                    # fused ph = num*rec and magic range reduction (1 ISA op);
                    # reads num straight from PSUM, freeing it per family.
                    if USE_MAGIC:
                        nc.vector._custom_dve(
                            MAGIC_PHASE,
                            out=G1[:, o : o + fs],
                            in0=num_ps[:, 0:fs],
                            in1=REC[:, o : o + fs],
                            s0=MAGIC - 0.25,
                            s1=MAGIC,
                        )
                    else:
                        PH = pool.tile([P, NALL], F32, tag="PH")
                        ph_eng = nc.gpsimd if PH_ENGINE == "pool" else nc.vector
                        ph_eng.tensor_tensor(
                            PH[:, o : o + fs], num_ps[:, 0:fs], REC[:, o : o + fs],
                            op=Alu.mult,
                        )
                        nr_eng = nc.gpsimd if NR_ENGINE == "pool" else nc.vector
                        nr_eng.tensor_scalar(
                            G1[:, o : o + fs], PH[:, o : o + fs], -0.25, MAGIC,
                            op0=Alu.add, op1=Alu.add,
                        )
                        nc.vector.scalar_tensor_tensor(
                            G1[:, o : o + fs], G1[:, o : o + fs], MAGIC,
                            PH[:, o : o + fs], op0=Alu.subtract, op1=Alu.subtract,
                        )

                # ---------- phase B (sliced for pipelining) ------------------
                if os.environ.get("K_FENCE", "0") == "1":
                    # tiny ACT op reading the last EE slice and writing C's
                    # first column: WAW-fences every sin behind the last exp,
                    # so the scheduler can't interleave trig/exp table loads.
                    nc.scalar.activation(
                        C[:, NALL - 1 : NALL], EE[:, NALL - 1 : NALL],
                        Act.Copy, bias=0.0, scale=1.0,
                    )
                _load_table(SET_TRIG)
                p2 = pool.tile([P, NALL], BF16, tag="p2")
                m2 = pool.tile([P, NALL], BF16, tag="m2")
                a2p = pool.tile([P, NALL], BF16, tag="a2p")
                a4p = pool.tile([P, NALL], BF16, tag="a4p")
                a2m = pool.tile([P, NALL], BF16, tag="a2m")
                a4m = pool.tile([P, NALL], BF16, tag="a4m")
                sinkd = pool.tile([P, NALL], BF16, tag="sinkd")
                sinkd2 = pool.tile([P, NALL], BF16, tag="sinkd2")
                p4s = pool.tile([P, NALL], BF16, tag="p4s")
                m4s = pool.tile([P, NALL], BF16, tag="m4s")
                # per-(branch, slice) partial accumulators, reduced at the end
                res24 = pool.tile([P, L, NSLICE], F32, tag="res24")
                CH = (NALL + NSLICE - 1) // NSLICE
                for si in range(NSLICE):
                    o = si * CH
                    fs = min(CH, NALL - o)
                    sl = slice(o, o + fs)

                    def acc(l, si=si):
                        return res24[:, l, si : si + 1]

                    # c = cos(2pi*ph) = sin(2pi*frn + pi/2), frn in [-.75,.25]
                    nc.scalar.activation(
                        C[:, sl], G1[:, sl], Act.Sin, bias=half_pi[:], scale=TWO_PI
                    )
                    nc.scalar.activation(
                        p2[:, sl], C[:, sl], Act.Square, bias=one_p[:], scale=1.0
                    )
                    nc.scalar.activation(
                        m2[:, sl], C[:, sl], Act.Square, bias=one_m[:], scale=1.0
                    )

                    def unit(impl, dst, in0, in1, outt, sink):
                        # outt = in0*in1 elementwise; dst = row-sum of outt
                        if impl == "amr":
                            nc.vector.affine_mul_reduce(
                                outt[:, sl], dst, in0[:, sl], in1[:, sl], 1.0, 0.0
                            )
                        else:
                            eng = nc.gpsimd if impl == "pool" else nc.vector
                            eng.tensor_tensor(
                                outt[:, sl], in0[:, sl], in1[:, sl], op=Alu.mult
                            )
                            nc.scalar.activation(
                                sink[:, sl], outt[:, sl], Act.Copy, bias=0.0,
                                scale=1.0, accum_out=dst,
                            )

                    def mul_sq_red(dst, in0, in1, sink, sqt):
                        if USE_MSR:
                            nc.vector._custom_dve(
                                MUL_SQ_RED, out=sink[:, sl], in0=in0[:, sl],
                                in1=in1[:, sl], s0=0.0, accum_out=dst,
                            )
                        else:
                            nc.vector.tensor_tensor(
                                sqt[:, sl], in1[:, sl], in1[:, sl], op=Alu.mult
                            )
                            nc.vector.affine_mul_reduce(
                                sink[:, sl], dst, in0[:, sl], sqt[:, sl], 1.0, 0.0
                            )

                    unit(UNITS[0], acc(0), EE, p2, a2p, sinkd)
                    unit(UNITS[1], acc(1), a2p, p2, a4p, sinkd)
                    mul_sq_red(acc(2), a4p, p2, sinkd, p4s)
                    unit(UNITS[2], acc(3), EE, m2, a2m, sinkd2)
                    unit(UNITS[3], acc(4), a2m, m2, a4m, sinkd2)
                    mul_sq_red(acc(5), a4m, m2, sinkd2, m4s)

                nc.vector.tensor_reduce(
                    res[:], res24[:], axis=mybir.AxisListType.X, op=Alu.add
                )

                resv = res[:].rearrange("p (s z) -> p s z", z=3)
                for zi in range(3):
                    nc.vector.tensor_scalar(
                        resv[:, :, zi], resv[:, :, zi], SCALES[zi], None, op0=Alu.mult
                    )
                nc.sync.dma_start(out.rearrange("b i l -> (b i) l"), res[:])

            for _rep in range(reps):
                _body()

    nc.compile()
    return nc


def _get_nc():
    global _NC
    if _NC is None:
        _NC = _build()
    return _NC


_RUNNER = None


def _get_runner():
    """Cached jitted SPMD runner (run_bass_kernel_spmd re-lowers per call;
    this builds the PJRT executable once and reuses it)."""
    global _RUNNER
    if _RUNNER is not None:
        return _RUNNER
    import jax
    from jax.sharding import Mesh, PartitionSpec
    from jax.experimental.shard_map import shard_map
    from concourse import bass2jax
    from concourse.bass2jax import _bass_exec_p, install_neuronx_cc_hook

    nc = _get_nc()
    install_neuronx_cc_hook()
    partition_name = nc.partition_id_tensor.name if nc.partition_id_tensor else None
    in_names, out_names, out_avals, zero_outs = [], [], [], []
    for alloc in nc.m.functions[0].allocations:
        if not isinstance(alloc, mybir.MemoryLocationSet):
            continue
        name = alloc.memorylocations[0].name
        if alloc.kind == "ExternalInput":
            if name != partition_name:
                in_names.append(name)
        elif alloc.kind == "ExternalOutput":
            shape = tuple(alloc.tensor_shape)
            dtype = mybir.dt.np(alloc.dtype)
            out_names.append(name)
            out_avals.append(jax.core.ShapedArray(shape, dtype))
            zero_outs.append(np.zeros(shape, dtype))
    all_names = in_names + out_names + ([partition_name] if partition_name else [])

    def one(*args):
        ops = list(args)
        if partition_name is not None:
            ops.append(bass2jax.partition_id_tensor())
        return tuple(
            _bass_exec_p.bind(
                *ops,
                out_avals=tuple(out_avals),
                in_names=tuple(all_names),
                out_names=tuple(out_names),
                lowering_input_output_aliases=(),
                sim_require_finite=True,
                sim_require_nnan=True,
                nc=nc,
            )
        )

    devices = jax.devices()[:NCORES]
    mesh = Mesh(np.asarray(devices), ("core",))
    specs = (PartitionSpec("core"),) * (len(in_names) + len(out_names))
    out_specs = (PartitionSpec("core"),) * len(out_names)
    fn = jax.jit(
        shard_map(one, mesh=mesh, in_specs=specs, out_specs=out_specs, check_rep=False),
        keep_unused=True,
    )
    concat_zeros = [
        np.zeros((NCORES * z.shape[0], *z.shape[1:]), z.dtype) for z in zero_outs
    ]
    _RUNNER = (fn, in_names, out_names, out_avals, concat_zeros)
    return _RUNNER


def kernel(d_cutoff, d, atom_coordinates):
    full = {
        "d_cutoff": np.ascontiguousarray(d_cutoff, dtype=np.float32),
        "d": np.ascontiguousarray(d, dtype=np.float32),
        "atom_coordinates": np.ascontiguousarray(atom_coordinates, dtype=np.float32),
    }
    fn, in_names, out_names, out_avals, concat_zeros = _get_runner()
    concat_in = [full[name] for name in in_names]  # [B,...] == concat of per-core [BPC,...]
    outs = fn(*concat_in, *concat_zeros)
    oi = out_names.index("out")
    return np.asarray(outs[oi]).reshape(B, M, L)


if __name__ == "__main__":
    rng = np.random.default_rng(0)
    inputs = {
        "d_cutoff": rng.uniform(0, 1, (B, M, M)).astype(np.float32),
        "d": rng.uniform(0, 1, (B, M, M)).astype(np.float32),
        "atom_coordinates": rng.standard_normal((B, M, 3)).astype(np.float32),
    }
    out = kernel(**inputs)
    print("kernel out shape:", out.shape, "sample:", out[0, 0])


# revision 56
# speedup vs baseline: 5.6823x; 5.6823x over previous
"""Trainium2 Bass kernel for nn_AngularSymmetry (B=16, M=64, L=6), 8-core data parallel.

Math (per batch b, output row i, summed over j,k in [0,64)):
  num[i,j,k] = nsq[i] - G[i,j] - G[i,k] + G[j,k]        (= vec_ij . vec_ik)
  rec = 1/(2pi*(d_ij*d_ik + 1e-5));  ph = num*rec        (= theta/2pi)
  c = cos(2pi*ph) via g = mod(ph+0.75, 1), c = sin(2pi*g - pi)
  E[i,j,k] = s_ij*s_ik*s_jk,  s = exp(-4 d^2)*d_cutoff   (log-domain via PE)
  res[i,l] = 2^(1-zeta_l) * sum_jk (1 + lambda_l*c)^zeta_l * E

Structure per core (2 batches, P=128 partition rows = (b,i)):
 - num built on PE with 3-way bf16-split G (hi/lo/lo2 ~ f32 accuracy, 4x
   cheaper than fp32 matmul); nsq rides data rows of the per-block c=9
   jk-matmul (rhs ones rows).
 - E = exp(lsE) where lsE = ls_ij + ls_ik + ls_jk accumulates on PE in
   bf16 (ls = ln(d_cutoff) - 4d^2; off-diagonal block families use
   ln(s + s^T) to fold the (j,k)/(k,j) pair weight).
 - den on DVE broadcast mult, reciprocal on ACT (eps folded into bias),
   range reduction via single tensor_scalar (add, mod).
 - branch products (1+-c)^zeta * E fused with their reductions via
   tensor_tensor_reduce / ACT copy-accum; squares via DVE ts-pow.
Only block-triangle families delta = kblk - jblk >= 0 are computed
(16x16 blocks); delta > 0 weights fold into the symmetrized ls2 table.
"""
import sys

sys.path.insert(0, "/opt/trn_rl_repo")
import contextlib

import numpy as np

import concourse.bass as bass
import concourse.tile as tile
from concourse import bacc, mybir
from concourse.bass_utils import run_bass_kernel_spmd

F32 = mybir.dt.float32
BF16 = mybir.dt.bfloat16
Alu = mybir.AluOpType
Act = mybir.ActivationFunctionType

# ---- custom DVE ops -------------------------------------------------------
# Registered into concourse.dve_ops at import; shas computed at build time.
import concourse.dve_ops as dve_ops
from concourse.dve_spec import C0, C1, Spec, Src0, Src1, Zero, lower, sq
from concourse.dve_spec import _has_src1 as _spec_has_src1
from concourse.dve_uop import DveOpSpec
from operator import add as _add


def _register_dve_op(name, spec, subdim=False):
    for op in dve_ops.OPS:
        if op.name == name:
            return op
    row = dve_ops._CUSTOM_DVE_ROW_BASE + len(dve_ops.OPS)
    dve_ops._SUB_OPCODE_FOR_NAME[name] = row
    shas = {}
    for ver in ("v3", "v4"):
        compiled = DveOpSpec(
            name=name, opcode=row, uops=lower(spec, ver=ver),
            rd1_en=_spec_has_src1(spec),
        )
        shas[ver] = compiled.sha(ver)
    op = dve_ops.DveOp(name, spec, subdim=subdim, uops_sha=shas)
    dve_ops.OPS.append(op)
    dve_ops.CUSTOM_DVE_SPECS[name] = spec
    return op


def _ref_magic_phase(in0, in1, s0, s1, imm2):
    ph = (in0.astype(np.float32) * in1).astype(np.float32)
    nr = (ph + np.float32(s0)).astype(np.float32)
    return ((nr - np.float32(s1)).astype(np.float32) - ph).astype(np.float32)


_mp_ph = Src0 * Src1
MAGIC_PHASE = _register_dve_op(
    "ANGSYM_MAGIC_PHASE",
    Spec(body=((_mp_ph + C0) - C1) - _mp_ph, reference=_ref_magic_phase),
)


def _ref_mul_sq_red(in0, in1, s0, s1, imm2):
    b = (in0.astype(np.float32) * (in1.astype(np.float32) ** 2)).astype(np.float32)
    return b, s0 + b.reshape(b.shape[0], -1).sum(axis=-1, keepdims=True)


MUL_SQ_RED = _register_dve_op(
    "ANGSYM_MUL_SQ_RED",
    Spec(
        body=Src0 * sq(Src1), accum=_add, accum_init=C0,
        reference=_ref_mul_sq_red,
    ),
)

B, M, L = 16, 64, 6
NCORES = 8
BPC = B // NCORES  # batches per core = 2
P = BPC * M  # 128 partitions
TWO_PI = float(2.0 * np.pi)
SQ2PI = float(np.sqrt(2.0 * np.pi))
EPS2PI = float(2.0 * np.pi * 1e-5)
MAGIC = 12582912.0  # 1.5 * 2^23 fp32 round-to-int

TB = 16
NT = M // TB  # 4
FAMS = [(d, NT - d) for d in range(NT)]  # (delta, nblocks)
FS = [nb * TB * TB for _, nb in FAMS]  # 1024, 768, 512, 256
OFF = [0, 1024, 1792, 2304]
NALL = 2560
SCALES = [1.0 / 2.0, 1.0 / 8.0, 1.0 / 128.0]  # 2^(1-zeta), zeta=2,4,8

# ---- tunables -------------------------------------------------------------
import os

DEN_ENGINE = os.environ.get("K_DEN", "pool")  # dve | pool
EPS_ENGINE = os.environ.get("K_EPS", "dve")  # dve | pool
PH_ENGINE = os.environ.get("K_PH", "pool")  # dve | pool
NSLICE = int(os.environ.get("K_NSLICE", "1"))  # phase-B slice count
# per-unit impl: amr (DVE fused) | pool (pool prod + ACT red) | dve (DVE tt
# prod + ACT red); units = (a2p, a4p, a2m, a4m)
UNITS = os.environ.get("K_UNITS", "amr,dve,pool,dve").split(",")

# HW-validated defaults: MSR custom op + explicit table loads ON;
# MAGIC_PHASE (loses fp32 intermediate rounding), TTR (crashes exec unit),
# and ts-accum (silently returns 0) OFF.
USE_MAGIC = os.environ.get("K_MAGIC", "0") == "1"
USE_MSR = os.environ.get("K_MSR", "1") == "1"
USE_TS_ACCUM = os.environ.get("K_TSACC", "0") == "1"
USE_TTR = os.environ.get("K_TTR", "0") == "1"
USE_LOADS = os.environ.get("K_LOADS", "1") == "1"

_NC = None


def _build(reps=1):
    nc = bacc.Bacc("TRN2", target_bir_lowering=False, debug=False, num_devices=NCORES)
    dcut = nc.dram_tensor("d_cutoff", [BPC, M, M], F32, kind="ExternalInput").ap()
    dd = nc.dram_tensor("d", [BPC, M, M], F32, kind="ExternalInput").ap()
    co = nc.dram_tensor("atom_coordinates", [BPC, M, 3], F32, kind="ExternalInput").ap()
    out = nc.dram_tensor("out", [BPC, M, L], F32, kind="ExternalOutput").ap()
    ghi_d = nc.dram_tensor("ghi_scratch", [BPC, M, M], BF16, kind="Internal").ap()
    glo_d = nc.dram_tensor("glo_scratch", [BPC, M, M], BF16, kind="Internal").ap()
    glo2_d = nc.dram_tensor("glo2_scratch", [BPC, M, M], BF16, kind="Internal").ap()
    ls_d = nc.dram_tensor("ls_scratch", [BPC, M, M], BF16, kind="Internal").ap()
    ls2_d = nc.dram_tensor("ls2_scratch", [BPC, M, M], BF16, kind="Internal").ap()
    # ind9 assembled via DRAM; 256-wide (512B/partition) for DMA step legality
    IW = 256
    ind9_d = nc.dram_tensor("ind9_scratch", [9, IW], BF16, kind="Internal").ap()
    gt9ones_d = nc.dram_tensor("gt9ones_scratch", [3, M, M], BF16, kind="Internal").ap()

    from concourse.hw_specs import get_activation_tables

    _tables = list(get_activation_tables(nc.m.arch).keys())
    SET_LNEXP = _tables.index("natural_log_exp_and_others")
    SET_TRIG = _tables.index("trig_and_small")

    # Restrict the auto table-load pass to our two sets (it greedily picks the
    # first set containing each function, thrashing between natural_log /
    # exp_and_others / trig). Indices into act_info.json must be preserved, so
    # non-preferred entries are blanked rather than removed.
    import concourse.bacc as _bacc_mod

    _orig_tables = get_activation_tables(nc.m.arch)

    def _filtered_tables(arch):
        full = _orig_tables
        keep = {"natural_log_exp_and_others", "trig_and_small"}
        return {k: (v if k in keep else set()) for k, v in full.items()}

    _bacc_mod.get_activation_tables = _filtered_tables

    def _load_table(set_id):
        if not USE_LOADS:
            return None
        inst = mybir.InstLoadActFuncSet(
            name=nc.get_next_instruction_name(), act_func_set_id=set_id,
            ins=[], outs=[],
        )
        return nc.scalar.add_instruction(inst)

    with tile.TileContext(nc) as tc:
        with contextlib.ExitStack() as ctx:
            pool = ctx.enter_context(tc.tile_pool(name="w", bufs=1))
            psp = ctx.enter_context(tc.tile_pool(name="ps", bufs=4, space="PSUM"))

            # ================= hoisted constants (input-independent) =========
            ones_t = pool.tile([P, P], F32, tag="ones_t")
            nc.vector.memset(ones_t[:], 1.0)
            idn = pool.tile([P, P], F32, tag="idn")
            nc.gpsimd.affine_select(
                idn[:], ones_t[:], pattern=[[1, P]], compare_op=Alu.is_equal,
                fill=0.0, channel_multiplier=-1,
            )
            ones3 = pool.tile([3, 1], F32, tag="ones3")
            nc.vector.memset(ones3[:], 1.0)
            ones2 = pool.tile([2, P], F32, tag="ones2")
            nc.vector.memset(ones2[:], 1.0)
            ind2a = pool.tile([2, P], F32, tag="ind2a")
            nc.gpsimd.affine_select(
                ind2a[:], ones2[:], pattern=[[1, P]], compare_op=Alu.is_ge,
                fill=0.0, base=0, channel_multiplier=-M,
            )
            ind2 = pool.tile([2, P], F32, tag="ind2")
            nc.gpsimd.affine_select(
                ind2[:], ind2a[:], pattern=[[-1, P]], compare_op=Alu.is_ge,
                fill=0.0, base=M - 1, channel_multiplier=M,
            )
            ind2b = pool.tile([2, IW], BF16, tag="ind2b")
            nc.vector.memset(ind2b[:], 0.0)
            nc.vector.tensor_copy(ind2b[:, 0:P], ind2[:])
            # ind9 rows 0-5 = (hi,lo,lo2)x(b0,b1) indicators (rows 6-8 set per
            # rep). Compute engines can only address partition starts
            # 0/32/64/96, so rows go through a DRAM scratch and come back in
            # one full-tile DMA read.
            ind9 = pool.tile([9, IW], BF16, tag="ind9")
            gt9 = pool.tile([9, M, M], BF16, tag="gt9")
            ones_row = pool.tile([1, M * M], BF16, tag="ones_row")
            nc.vector.memset(ones_row[:], 1.0)
            for s in range(3):
                nc.sync.dma_start(ind9_d[2 * s : 2 * s + 2], ind2b[:])
            for s in range(3):
                nc.sync.dma_start(
                    gt9ones_d[s : s + 1].rearrange("a j k -> a (j k)"), ones_row[:]
                )
            nc.sync.dma_start(gt9[6:9], gt9ones_d)
            half_pi = pool.tile([P, 1], F32, tag="half_pi")
            nc.vector.memset(half_pi[:], float(np.pi / 2.0))
            one_p = pool.tile([P, 1], F32, tag="one_p")
            nc.vector.memset(one_p[:], 1.0)
            one_m = pool.tile([P, 1], F32, tag="one_m")
            nc.vector.memset(one_m[:], -1.0)
            # Sx indicator tables per family (stacked twice for hi/lo lhs)
            sx2_t = {}
            sxb_t = {}
            for fi, (delta, nb) in enumerate(FAMS):
                fs = FS[fi]
                sxf = pool.tile([M, fs], F32, tag=f"sxf{delta}")
                jv = (
                    idn[0:M, 0 : nb * TB]
                    .rearrange("c (n j) -> c n j", j=TB)
                    .unsqueeze(3)
                    .broadcast_to([M, nb, TB, TB])
                )
                kv = (
                    idn[0:M, delta * TB : (delta + nb) * TB]
                    .rearrange("c (n k) -> c n k", k=TB)
                    .unsqueeze(2)
                    .broadcast_to([M, nb, TB, TB])
                )
                nc.gpsimd.tensor_tensor(
                    sxf[:, 0:fs].rearrange("c (n j k) -> c n j k", j=TB, k=TB),
                    jv, kv, op=Alu.add,
                )
                sx2 = pool.tile([P, fs], BF16, tag=f"sx2{delta}")
                nc.vector.tensor_copy(sx2[0:M, :], sxf[:])
                nc.vector.tensor_copy(sx2[M:P, :], sxf[:])
                sx2_t[fi] = sx2
                sxb_t[fi] = sx2[0:M, :]

            def _body():
                _load_table(SET_LNEXP)
                # ---------- input DMAs ----------
                d_t = pool.tile([P, M], F32, tag="d_t")
                dc_t = pool.tile([P, M], F32, tag="dc_t")
                nc.sync.dma_start(d_t[:], dd.rearrange("b i j -> (b i) j"))
                nc.sync.dma_start(dc_t[:], dcut.rearrange("b i j -> (b i) j"))
                ct3 = pool.tile([3, P], F32, tag="ct3")
                nc.sync.dma_start(ct3[:], co.rearrange("b i d -> d (b i)"))

                # ---------- den (independent of PE prep) ----------
                ut = pool.tile([P, M], F32, tag="ut")
                nc.vector.tensor_scalar(ut[:], d_t[:], SQ2PI, None, op0=Alu.mult)
                DEN = pool.tile([P, NALL], F32, tag="DEN")

                def jbc(t, delta, nb):
                    v = t[:, 0 : nb * TB].rearrange("p (n j) -> p n j", j=TB)
                    return v.unsqueeze(3).broadcast_to([P, nb, TB, TB])

                def kbc(t, delta, nb):
                    v = t[:, delta * TB : (delta + nb) * TB].rearrange(
                        "p (n k) -> p n k", k=TB
                    )
                    return v.unsqueeze(2).broadcast_to([P, nb, TB, TB])

                def g4(big, fi):
                    delta, nb = FAMS[fi]
                    return big[:, OFF[fi] : OFF[fi] + FS[fi]].rearrange(
                        "p (n j k) -> p n j k", j=TB, k=TB
                    )

                den_eng = nc.vector if DEN_ENGINE == "dve" else nc.gpsimd
                for fi, (delta, nb) in enumerate(FAMS):
                    den_eng.tensor_tensor(
                        g4(DEN, fi), jbc(ut, delta, nb), kbc(ut, delta, nb), op=Alu.mult
                    )

                # ---------- Gram, nsq ----------
                gram_ps = psp.tile([P, P], F32, tag="ps")
                nc.tensor.matmul(gram_ps[:], ct3[:], ct3[:], start=True, stop=True)
                g_sb = pool.tile([P, M], F32, tag="g_sb")
                nc.scalar.copy(g_sb[0:M, :], gram_ps[0:M, 0:M])
                nc.scalar.copy(g_sb[M:P, :], gram_ps[M:P, M:P])

                sq3t = pool.tile([3, P], F32, tag="sq3t")
                nc.scalar.square(sq3t[:], ct3[:])
                nsq_ps = psp.tile([1, P], F32, tag="ps")
                nc.tensor.matmul(nsq_ps[:], ones3[:], sq3t[:], start=True, stop=True)
                nsq_row = pool.tile([1, P], F32, tag="nsq_row")
                nc.scalar.copy(nsq_row[:], nsq_ps[:])

                # ---------- -G transposed + 3-way bf16 split (lhs of num) ----
                wneg = pool.tile([P, M], F32, tag="wneg")
                nc.scalar.activation(wneg[:], g_sb[:], Act.Copy, bias=0.0, scale=-1.0)
                wG_ps = psp.tile([M, P], F32, tag="ps")
                nc.tensor.transpose(wG_ps[:], wneg[:], idn[:])

                numA = pool.tile([P, P], BF16, tag="numA")  # rows 0:64 hi, 64:128 lo
                numB = pool.tile([M, P], BF16, tag="numB")  # lo2
                nc.vector.tensor_copy(numA[0:M, :], wG_ps[:])
                tmpG = pool.tile([M, P], F32, tag="tmpG")
                nc.vector.scalar_tensor_tensor(
                    tmpG[:], wG_ps[:], 0.0, numA[0:M, :], op0=Alu.bypass, op1=Alu.subtract
                )
                gloT = pool.tile([M, P], BF16, tag="gloT")
                nc.vector.tensor_copy(gloT[:], tmpG[:])
                nc.vector.tensor_copy(numA[M:P, :], gloT[:])
                nc.vector.scalar_tensor_tensor(
                    numB[:], tmpG[:], 0.0, gloT[:], op0=Alu.bypass, op1=Alu.subtract
                )

                # ind9 rows 6-8: nsq 3-way bf16 split (data rows, via DRAM)
                nsq_h = pool.tile([1, IW], BF16, tag="nsq_h")
                nsq_l = pool.tile([1, IW], BF16, tag="nsq_l")
                nsq_l2 = pool.tile([1, IW], BF16, tag="nsq_l2")
                tmpn = pool.tile([1, P], F32, tag="tmpn")
                nc.vector.memset(nsq_h[:], 0.0)
                nc.vector.memset(nsq_l[:], 0.0)
                nc.vector.memset(nsq_l2[:], 0.0)
                nc.vector.tensor_copy(nsq_h[:, 0:P], nsq_row[:])
                nc.vector.scalar_tensor_tensor(
                    tmpn[:], nsq_row[:], 0.0, nsq_h[:, 0:P], op0=Alu.bypass,
                    op1=Alu.subtract,
                )
                nc.vector.tensor_copy(nsq_l[:, 0:P], tmpn[:])
                nc.vector.scalar_tensor_tensor(
                    nsq_l2[:, 0:P], tmpn[:], 0.0, nsq_l[:, 0:P], op0=Alu.bypass,
                    op1=Alu.subtract,
                )
                nc.sync.dma_start(ind9_d[6:7], nsq_h[:])
                nc.sync.dma_start(ind9_d[7:8], nsq_l[:])
                nc.sync.dma_start(ind9_d[8:9], nsq_l2[:])
                nc.sync.dma_start(ind9[:], ind9_d)

                # ---------- gjk split staging (rhs table for num-jk matmul) ----
                ghi_sb = pool.tile([P, M], BF16, tag="ghi_sb")
                glo_sb = pool.tile([P, M], BF16, tag="glo_sb")
                glo2_sb = pool.tile([P, M], BF16, tag="glo2_sb")
                nc.vector.tensor_copy(ghi_sb[:], g_sb[:])
                tmpg2 = pool.tile([P, M], F32, tag="tmpg2")
                nc.vector.scalar_tensor_tensor(
                    tmpg2[:], g_sb[:], 0.0, ghi_sb[:], op0=Alu.bypass, op1=Alu.subtract
                )
                nc.vector.tensor_copy(glo_sb[:], tmpg2[:])
                nc.vector.scalar_tensor_tensor(
                    glo2_sb[:], tmpg2[:], 0.0, glo_sb[:], op0=Alu.bypass, op1=Alu.subtract
                )
                nc.sync.dma_start(ghi_d.rearrange("b j k -> (b j) k"), ghi_sb[:])
                nc.sync.dma_start(glo_d.rearrange("b j k -> (b j) k"), glo_sb[:])
                nc.sync.dma_start(glo2_d.rearrange("b j k -> (b j) k"), glo2_sb[:])
                nc.sync.dma_start(gt9[0:2], ghi_d)
                nc.sync.dma_start(gt9[2:4], glo_d)
                nc.sync.dma_start(gt9[4:6], glo2_d)

                # ---------- ls prep (log-domain E) ----------
                lndc = pool.tile([P, M], F32, tag="lndc")
                nc.scalar.activation(lndc[:], dc_t[:], Act.Ln, bias=0.0, scale=1.0)
                d2 = pool.tile([P, M], F32, tag="d2")
                nc.scalar.square(d2[:], d_t[:])
                ls = pool.tile([P, M], F32, tag="ls")
                nc.vector.scalar_tensor_tensor(
                    ls[:], d2[:], -4.0, lndc[:], op0=Alu.mult, op1=Alu.add
                )
                nc.vector.tensor_scalar(ls[:], ls[:], -60.0, None, op0=Alu.max)
                s_t = pool.tile([P, M], F32, tag="s_t")
                nc.scalar.activation(s_t[:], ls[:], Act.Exp, bias=0.0, scale=1.0)

                # lsx = ls transposed [64, P] bf16 (lhs of lsE-main matmul)
                ls_ps = psp.tile([M, P], F32, tag="ps")
                nc.tensor.transpose(ls_ps[:], ls[:], idn[:])
                lsx = pool.tile([M, P], BF16, tag="lsx")
                nc.vector.tensor_copy(lsx[:], ls_ps[:])

                # symmetrized ls2 = ln(s + s^T) per batch
                sfull = pool.tile([P, P], F32, tag="sfull")
                nc.vector.memset(sfull[:], 0.0)
                nc.vector.tensor_copy(sfull[0:M, 0:M], s_t[0:M, :])
                nc.vector.tensor_copy(sfull[M:P, M:P], s_t[M:P, :])
                sfT_ps = psp.tile([P, P], F32, tag="ps")
                nc.tensor.transpose(sfT_ps[:], sfull[:], idn[:])
                qsf = pool.tile([P, P], F32, tag="qsf")
                nc.vector.tensor_tensor(qsf[:], sfull[:], sfT_ps[:], op=Alu.add)
                ls2 = pool.tile([P, M], F32, tag="ls2")
                nc.scalar.activation(ls2[0:M, :], qsf[0:M, 0:M], Act.Ln, bias=0.0, scale=1.0)
                nc.scalar.activation(ls2[M:P, :], qsf[M:P, M:P], Act.Ln, bias=0.0, scale=1.0)

                ls_b = pool.tile([P, M], BF16, tag="ls_b")
                nc.vector.tensor_copy(ls_b[:], ls[:])
                ls2_b = pool.tile([P, M], BF16, tag="ls2_b")
                nc.vector.tensor_copy(ls2_b[:], ls2[:])
                nc.sync.dma_start(ls_d.rearrange("b j k -> (b j) k"), ls_b[:])
                nc.sync.dma_start(ls2_d.rearrange("b j k -> (b j) k"), ls2_b[:])
                lst2 = pool.tile([2, M, M], BF16, tag="lst2")
                nc.sync.dma_start(lst2[:], ls_d)
                ls2t2 = pool.tile([2, M, M], BF16, tag="ls2t2")
                nc.sync.dma_start(ls2t2[:], ls2_d)

                # ---------- big tiles ----------
                EE = pool.tile([P, NALL], BF16, tag="EE")
                G1 = pool.tile([P, NALL], F32, tag="G1")
                C = pool.tile([P, NALL], BF16, tag="C")
                res = pool.tile([P, L], F32, tag="res")

                # eps fold: DEN += 2pi*1e-5, then 18-bit reciprocal (1 ISA op)
                eps_eng = nc.vector if EPS_ENGINE == "dve" else nc.gpsimd
                eps_eng.tensor_scalar(DEN[:], DEN[:], EPS2PI, None, op0=Alu.add)
                REC = pool.tile([P, NALL], F32, tag="REC")
                nc.vector.reciprocal_approx_fast(REC[:], DEN[:])

                # ---------- phase A: per-family matmuls + Eh-exp + ph ----------
                for fi, (delta, nb) in enumerate(FAMS):
                    fs = FS[fi]
                    o = OFF[fi]
                    num_ps = psp.tile([P, 1024], F32, tag="ps")
                    lse_ps = psp.tile([P, 1024], F32, tag="ps")
                    # lsE first so the ACT exp is ready before this family's
                    # divide chain — keeps Sin after the last Exp on ACT.
                    # Matmul outputs are chunked to 512 cols (one PSUM bank).
                    for c0 in range(0, fs, 512):
                        c1 = min(c0 + 512, fs)
                        nc.tensor.matmul(
                            lse_ps[:, c0:c1], lsx[:], sxb_t[fi][:, c0:c1],
                            start=True, stop=False,
                        )
                    lsrc = lst2 if delta == 0 else ls2t2
                    for n in range(nb):
                        j0 = n * TB
                        k0 = (n + delta) * TB
                        cols = TB * TB
                        nc.tensor.matmul(
                            lse_ps[:, n * cols : (n + 1) * cols],
                            ind2b[0:2, 0:P], lsrc[:, j0 : j0 + TB, k0 : k0 + TB],
                            start=False, stop=True,
                        )
                    for c0 in range(0, fs, 512):
                        c1 = min(c0 + 512, fs)
                        nc.tensor.matmul(
                            num_ps[:, c0:c1], numA[:], sx2_t[fi][:, c0:c1],
                            start=True, stop=False,
                        )
                        nc.tensor.matmul(
                            num_ps[:, c0:c1], numB[:], sxb_t[fi][:, c0:c1],
                            start=False, stop=False,
                        )
                    for n in range(nb):
                        j0 = n * TB
                        k0 = (n + delta) * TB
                        cols = TB * TB
                        nc.tensor.matmul(
                            num_ps[:, n * cols : (n + 1) * cols],
                            ind9[0:9, 0:P], gt9[:, j0 : j0 + TB, k0 : k0 + TB],
                            start=False, stop=True,
                        )
                    nc.scalar.activation(
                        EE[:, o : o + fs], lse_ps[:, 0:fs], Act.Exp, bias=0.0, scale=1.0
                    )
                    # fused ph = num*rec and magic range reduction (1 ISA op);
                    # reads num straight from PSUM, freeing it per family.
                    if USE_MAGIC:
                        nc.vector._custom_dve(
                            MAGIC_PHASE,
                            out=G1[:, o : o + fs],
                            in0=num_ps[:, 0:fs],
                            in1=REC[:, o : o + fs],
                            s0=MAGIC - 0.25,
                            s1=MAGIC,
                        )
                    else:
                        PH = pool.tile([P, NALL], F32, tag="PH")
                        nc.vector.tensor_tensor(
                            PH[:, o : o + fs], num_ps[:, 0:fs], REC[:, o : o + fs],
                            op=Alu.mult,
                        )
                        nc.vector.tensor_scalar(
                            G1[:, o : o + fs], PH[:, o : o + fs], -0.25, MAGIC,
                            op0=Alu.add, op1=Alu.add,
                        )
                        nc.vector.scalar_tensor_tensor(
                            G1[:, o : o + fs], G1[:, o : o + fs], MAGIC,
                            PH[:, o : o + fs], op0=Alu.subtract, op1=Alu.subtract,
                        )

                # ---------- phase B (sliced for pipelining) ------------------
                if os.environ.get("K_FENCE", "0") == "1":
                    # tiny ACT op reading the last EE slice and writing C's
                    # first column: WAW-fences every sin behind the last exp,
                    # so the scheduler can't interleave trig/exp table loads.
                    nc.scalar.activation(
                        C[:, NALL - 1 : NALL], EE[:, NALL - 1 : NALL],
                        Act.Copy, bias=0.0, scale=1.0,
                    )
                _load_table(SET_TRIG)
                p2 = pool.tile([P, NALL], BF16, tag="p2")
                m2 = pool.tile([P, NALL], BF16, tag="m2")
                a2p = pool.tile([P, NALL], BF16, tag="a2p")
                a4p = pool.tile([P, NALL], BF16, tag="a4p")
                a2m = pool.tile([P, NALL], BF16, tag="a2m")
                a4m = pool.tile([P, NALL], BF16, tag="a4m")
                sinkd = pool.tile([P, NALL], BF16, tag="sinkd")
                sinkd2 = pool.tile([P, NALL], BF16, tag="sinkd2")
                p4s = pool.tile([P, NALL], BF16, tag="p4s")
                m4s = pool.tile([P, NALL], BF16, tag="m4s")
                # per-(branch, slice) partial accumulators, reduced at the end
                res24 = pool.tile([P, L, NSLICE], F32, tag="res24")
                CH = (NALL + NSLICE - 1) // NSLICE
                for si in range(NSLICE):
                    o = si * CH
                    fs = min(CH, NALL - o)
                    sl = slice(o, o + fs)

                    def acc(l, si=si):
                        return res24[:, l, si : si + 1]

                    # c = cos(2pi*ph) = sin(2pi*frn + pi/2), frn in [-.75,.25]
                    nc.scalar.activation(
                        C[:, sl], G1[:, sl], Act.Sin, bias=half_pi[:], scale=TWO_PI
                    )
                    nc.scalar.activation(
                        p2[:, sl], C[:, sl], Act.Square, bias=one_p[:], scale=1.0
                    )
                    nc.scalar.activation(
                        m2[:, sl], C[:, sl], Act.Square, bias=one_m[:], scale=1.0
                    )

                    def unit(impl, dst, in0, in1, outt, sink):
                        # outt = in0*in1 elementwise; dst = row-sum of outt
                        if impl == "amr":
                            nc.vector.affine_mul_reduce(
                                outt[:, sl], dst, in0[:, sl], in1[:, sl], 1.0, 0.0
                            )
                        else:
                            eng = nc.gpsimd if impl == "pool" else nc.vector
                            eng.tensor_tensor(
                                outt[:, sl], in0[:, sl], in1[:, sl], op=Alu.mult
                            )
                            nc.scalar.activation(
                                sink[:, sl], outt[:, sl], Act.Copy, bias=0.0,
                                scale=1.0, accum_out=dst,
                            )

                    def mul_sq_red(dst, in0, in1, sink, sqt):
                        if USE_MSR:
                            nc.vector._custom_dve(
                                MUL_SQ_RED, out=sink[:, sl], in0=in0[:, sl],
                                in1=in1[:, sl], s0=0.0, accum_out=dst,
                            )
                        else:
                            nc.vector.tensor_tensor(
                                sqt[:, sl], in1[:, sl], in1[:, sl], op=Alu.mult
                            )
                            nc.vector.affine_mul_reduce(
                                sink[:, sl], dst, in0[:, sl], sqt[:, sl], 1.0, 0.0
                            )

                    unit(UNITS[0], acc(0), EE, p2, a2p, sinkd)
                    unit(UNITS[1], acc(1), a2p, p2, a4p, sinkd)
                    mul_sq_red(acc(2), a4p, p2, sinkd, p4s)
                    unit(UNITS[2], acc(3), EE, m2, a2m, sinkd2)
                    unit(UNITS[3], acc(4), a2m, m2, a4m, sinkd2)
                    mul_sq_red(acc(5), a4m, m2, sinkd2, m4s)

                nc.vector.tensor_reduce(
                    res[:], res24[:], axis=mybir.AxisListType.X, op=Alu.add
                )

                resv = res[:].rearrange("p (s z) -> p s z", z=3)
                for zi in range(3):
                    nc.vector.tensor_scalar(
                        resv[:, :, zi], resv[:, :, zi], SCALES[zi], None, op0=Alu.mult
                    )
                nc.sync.dma_start(out.rearrange("b i l -> (b i) l"), res[:])

            for _rep in range(reps):
                _body()

    nc.compile()
    return nc


def _get_nc():
    global _NC
    if _NC is None:
        _NC = _build()
    return _NC


_RUNNER = None


def _get_runner():
    """Cached jitted SPMD runner (run_bass_kernel_spmd re-lowers per call;
    this builds the PJRT executable once and reuses it)."""
    global _RUNNER
    if _RUNNER is not None:
        return _RUNNER
    import jax
    from jax.sharding import Mesh, PartitionSpec
    from jax.experimental.shard_map import shard_map
    from concourse import bass2jax
    from concourse.bass2jax import _bass_exec_p, install_neuronx_cc_hook

    nc = _get_nc()
    install_neuronx_cc_hook()
    partition_name = nc.partition_id_tensor.name if nc.partition_id_tensor else None
    in_names, out_names, out_avals, zero_outs = [], [], [], []
    for alloc in nc.m.functions[0].allocations:
        if not isinstance(alloc, mybir.MemoryLocationSet):
            continue
        name = alloc.memorylocations[0].name
        if alloc.kind == "ExternalInput":
            if name != partition_name:
                in_names.append(name)
        elif alloc.kind == "ExternalOutput":
            shape = tuple(alloc.tensor_shape)
            dtype = mybir.dt.np(alloc.dtype)
            out_names.append(name)
            out_avals.append(jax.core.ShapedArray(shape, dtype))
            zero_outs.append(np.zeros(shape, dtype))
    all_names = in_names + out_names + ([partition_name] if partition_name else [])

    def one(*args):
        ops = list(args)
        if partition_name is not None:
            ops.append(bass2jax.partition_id_tensor())
        return tuple(
            _bass_exec_p.bind(
                *ops,
                out_avals=tuple(out_avals),
                in_names=tuple(all_names),
                out_names=tuple(out_names),
                lowering_input_output_aliases=(),
                sim_require_finite=True,
                sim_require_nnan=True,
                nc=nc,
            )
        )

    devices = jax.devices()[:NCORES]
    mesh = Mesh(np.asarray(devices), ("core",))
    specs = (PartitionSpec("core"),) * (len(in_names) + len(out_names))
    out_specs = (PartitionSpec("core"),) * len(out_names)
    fn = jax.jit(
        shard_map(one, mesh=mesh, in_specs=specs, out_specs=out_specs, check_rep=False),
        keep_unused=True,
    )
    concat_zeros = [
        np.zeros((NCORES * z.shape[0], *z.shape[1:]), z.dtype) for z in zero_outs
    ]
    _RUNNER = (fn, in_names, out_names, out_avals, concat_zeros)
    return _RUNNER


def kernel(d_cutoff, d, atom_coordinates):
    full = {
        "d_cutoff": np.ascontiguousarray(d_cutoff, dtype=np.float32),
        "d": np.ascontiguousarray(d, dtype=np.float32),
        "atom_coordinates": np.ascontiguousarray(atom_coordinates, dtype=np.float32),
    }
    fn, in_names, out_names, out_avals, concat_zeros = _get_runner()
    concat_in = [full[name] for name in in_names]  # [B,...] == concat of per-core [BPC,...]
    outs = fn(*concat_in, *concat_zeros)
    oi = out_names.index("out")
    return np.asarray(outs[oi]).reshape(B, M, L)


if __name__ == "__main__":
    rng = np.random.default_rng(0)
    inputs = {
        "d_cutoff": rng.uniform(0, 1, (B, M, M)).astype(np.float32),
        "d": rng.uniform(0, 1, (B, M, M)).astype(np.float32),
        "atom_coordinates": rng.standard_normal((B, M, 3)).astype(np.float32),
    }
    out = kernel(**inputs)
    print("kernel out shape:", out.shape, "sample:", out[0, 0])
